# revision 5
# baseline (speedup 1.0000x reference)
"""Trainium2 Bass kernel for nn_CR8_reg_cond_mul_5 (moe_routing).

Pipeline per pixel (B=16, C=128, H=1, W=8192; N = 131072 pixels):
  classifier: h = lrelu(bn(cl1 @ x)); x2 = lrelu(cl2 @ h); L = cl3 @ x2
  inds = argmax(L[:128]);  mask = lrelu(L[128])
  regression: r = lrelu(bn(reg1 @ x)); cat = [r; h]
  y = lrelu(cat @ w2[inds//16] + b2[inds//16])
  reg = y . w3[inds,:,0] + b3[inds];  x_real = (inds + reg) / 128

Sharding: data-parallel over batch; core c handles batches {2c, 2c+1}
(16384 pixels), weights replicated. No collectives.

On-chip strategy (channel-major [C, pixels] tiles of 512 px):
  - classifier matmuls in native fp32 (argmax needs fp32-grade logits);
  - argmax via PE transpose -> DVE max-reduce -> exact-equality one-hot
    -> PE transpose back to channel-major;
  - CondMul: all 8 experts computed as 2 expert-packed [128,...] bf16
    matmuls; per-pixel expert/class selection done by a single bf16
    matmul with a precomputed block-masked w3 table against the one-hot
    (folds the expert mask, the w3 gather and b3 gather into matmuls);
  - final dot + index + biases accumulated in one PSUM row.
"""
import numpy as np
import ml_dtypes

import concourse.bass as bass
import concourse.bacc as bacc
import concourse.mybir as mybir
import concourse.tile as tile
from concourse.bass_utils import run_bass_kernel_spmd

F32 = mybir.dt.float32
BF16 = mybir.dt.bfloat16
AF = mybir.ActivationFunctionType
ALU = mybir.AluOpType
AX = mybir.AxisListType

B, C, W = 16, 128, 8192
NCORES = 8
BPC = B // NCORES          # batches per core
TILE = 512
NTILES = W // TILE
CLASSES = 128
EPS = 1e-5

_CACHE = {}


def _build_nc():
    nc = bacc.Bacc("TRN2", target_bir_lowering=False, debug=False)

    x_d = nc.dram_tensor("x", [BPC, C, W], F32, kind="ExternalInput")
    w1t_d = nc.dram_tensor("w1t", [128, 128], F32, kind="ExternalInput")
    s1_d = nc.dram_tensor("s1", [128, 1], F32, kind="ExternalInput")
    b1_d = nc.dram_tensor("b1", [128, 1], F32, kind="ExternalInput")
    w2ct_d = nc.dram_tensor("w2ct", [128, 128], F32, kind="ExternalInput")
    b2c_d = nc.dram_tensor("b2c", [128, 1], F32, kind="ExternalInput")
    w3ct_d = nc.dram_tensor("w3ct", [128, 128], F32, kind="ExternalInput")
    b3c_d = nc.dram_tensor("b3c", [128, 1], F32, kind="ExternalInput")
    wlast_d = nc.dram_tensor("wlast", [128, 1], BF16, kind="ExternalInput")
    maskb_d = nc.dram_tensor("maskb", [1, 1], F32, kind="ExternalInput")
    r1t_d = nc.dram_tensor("r1t", [128, 128], BF16, kind="ExternalInput")
    sr_d = nc.dram_tensor("sr", [128, 1], F32, kind="ExternalInput")
    br_d = nc.dram_tensor("br", [128, 1], F32, kind="ExternalInput")
    w2p_d = nc.dram_tensor("w2p", [2, 2, 128, 128], BF16, kind="ExternalInput")
    b2s_d = nc.dram_tensor("b2s", [2, 128, 1], F32, kind="ExternalInput")
    w3sel_d = nc.dram_tensor("w3sel", [2, 128, 128], BF16, kind="ExternalInput")
    iotac_d = nc.dram_tensor("iotac", [128, 1], BF16, kind="ExternalInput")
    b3col_d = nc.dram_tensor("b3col", [128, 1], BF16, kind="ExternalInput")
    onesc_d = nc.dram_tensor("onesc", [128, 1], BF16, kind="ExternalInput")
    idn32_d = nc.dram_tensor("idn32", [128, 128], F32, kind="ExternalInput")
    idnbf_d = nc.dram_tensor("idnbf", [128, 128], BF16, kind="ExternalInput")

    xr_d = nc.dram_tensor("xr", [BPC, W], F32, kind="ExternalOutput")
    mask_d = nc.dram_tensor("mask", [BPC, W], F32, kind="ExternalOutput")

    with tile.TileContext(nc) as tc:
        with (
            tc.tile_pool(name="consts", bufs=1) as cp,
            tc.tile_pool(name="xin", bufs=3) as xp,
            tc.tile_pool(name="work", bufs=3) as wp,
            tc.tile_pool(name="psmm", bufs=4, space="PSUM") as pm,
            tc.tile_pool(name="psoh", bufs=1, space="PSUM") as po,
            tc.tile_pool(name="psrow", bufs=1, space="PSUM") as pr,
        ):
            # ---- constants into SBUF once ----
            def cload(dram_ap, shape, dt, tag=None):
                t = cp.tile(shape, dt, tag=tag or dram_ap.tensor.name)
                nc.sync.dma_start(t[:], dram_ap)
                return t

            w1t = cload(w1t_d[:], [128, 128], F32)
            s1 = cload(s1_d[:], [128, 1], F32)
            b1 = cload(b1_d[:], [128, 1], F32)
            w2ct = cload(w2ct_d[:], [128, 128], F32)
            b2c = cload(b2c_d[:], [128, 1], F32)
            w3ct = cload(w3ct_d[:], [128, 128], F32)
            b3c = cload(b3c_d[:], [128, 1], F32)
            wlast = cload(wlast_d[:], [128, 1], BF16)
            maskb = cload(maskb_d[:], [1, 1], F32)
            r1t = cload(r1t_d[:], [128, 128], BF16)
            sr = cload(sr_d[:], [128, 1], F32)
            br = cload(br_d[:], [128, 1], F32)
            w2p = [[cload(w2p_d[g, kh], [128, 128], BF16, tag=f"w2p{g}{kh}") for kh in range(2)]
                   for g in range(2)]
            b2s = [cload(b2s_d[g], [128, 1], F32, tag=f"b2s{g}") for g in range(2)]
            w3sel = [cload(w3sel_d[g], [128, 128], BF16, tag=f"w3sel{g}") for g in range(2)]
            iotac = cload(iotac_d[:], [128, 1], BF16)
            b3col = cload(b3col_d[:], [128, 1], BF16)
            onesc = cload(onesc_d[:], [128, 1], BF16)
            idn32 = cload(idn32_d[:], [128, 128], F32)
            idnbf = cload(idnbf_d[:], [128, 128], BF16)

            for b in range(BPC):
                for t in range(NTILES):
                    w0 = t * TILE
                    x_t = xp.tile([128, TILE], F32, tag="x")
                    nc.sync.dma_start(x_t[:], x_d[b, :, w0:w0 + TILE])
                    xb_t = wp.tile([128, TILE], BF16, tag="xb")
                    nc.vector.tensor_copy(xb_t[:], x_t[:])

                    # classifier layer 1 (fp32) + fused bnorm + lrelu
                    ps_h = pm.tile([128, TILE], F32, tag="mm")
                    nc.tensor.matmul(ps_h[:], w1t[:], x_t[:], start=True, stop=True)
                    h_t = wp.tile([128, TILE], F32, tag="h")
                    nc.scalar.activation(h_t[:], ps_h[:], AF.Lrelu,
                                         bias=b1[:], scale=s1[:], alpha=0.01)
                    hb_t = wp.tile([128, TILE], BF16, tag="hb")
                    nc.vector.tensor_copy(hb_t[:], h_t[:])

                    # regression layer 1 (bf16) + fused bnorm + lrelu
                    ps_r = pm.tile([128, TILE], F32, tag="mm")
                    nc.tensor.matmul(ps_r[:], r1t[:], xb_t[:], start=True, stop=True)
                    rb_t = wp.tile([128, TILE], BF16, tag="rb")
                    nc.scalar.activation(rb_t[:], ps_r[:], AF.Lrelu,
                                         bias=br[:], scale=sr[:], alpha=0.01)

                    # classifier layer 2 (fp32) + lrelu
                    ps_x2 = pm.tile([128, TILE], F32, tag="mm")
                    nc.tensor.matmul(ps_x2[:], w2ct[:], h_t[:], start=True, stop=True)
                    x2_t = wp.tile([128, TILE], F32, tag="x2")
                    nc.scalar.activation(x2_t[:], ps_x2[:], AF.Lrelu,
                                         bias=b2c[:], alpha=0.01)
                    x2b_t = wp.tile([128, TILE], BF16, tag="x2b")
                    nc.vector.tensor_copy(x2b_t[:], x2_t[:])

                    # classifier layer 3 logits (fp32) + bias
                    ps_l = pm.tile([128, TILE], F32, tag="mm")
                    nc.tensor.matmul(ps_l[:], w3ct[:], x2_t[:], start=True, stop=True)
                    l_t = wp.tile([128, TILE], F32, tag="l")
                    nc.scalar.activation(l_t[:], ps_l[:], AF.Identity, bias=b3c[:])

                    # mask channel (bf16 row matmul) + lrelu
                    ps_m = pr.tile([1, TILE], F32, tag="mrow")
                    nc.tensor.matmul(ps_m[:], wlast[:], x2b_t[:], start=True, stop=True)
                    mrow = wp.tile([1, TILE], F32, tag="mrow_sb")
                    nc.scalar.activation(mrow[:], ps_m[:], AF.Lrelu,
                                         bias=maskb[:], alpha=0.01)
                    nc.sync.dma_start(mask_d[b:b + 1, w0:w0 + TILE], mrow[:])

                    # transpose logits to pixel-major
                    ps_lt = pm.tile([128, TILE], F32, tag="mm")
                    for j in range(4):
                        nc.tensor.transpose(ps_lt[:, j * 128:(j + 1) * 128],
                                            l_t[:, j * 128:(j + 1) * 128], idn32[:])
                    lt3 = ps_lt[:].rearrange("p (b c) -> p b c", c=128)

                    # per-pixel max over classes, exact-equality one-hot
                    maxv = wp.tile([128, 4], F32, tag="maxv")
                    nc.vector.tensor_reduce(maxv[:], lt3, AX.X, ALU.max)
                    eq_t = wp.tile([128, TILE], BF16, tag="eq")
                    eq3 = eq_t[:].rearrange("p (b c) -> p b c", c=128)
                    maxb = maxv[:].unsqueeze(-1).broadcast_to([128, 4, 128])
                    nc.vector.tensor_tensor(eq3, lt3, maxb, ALU.is_equal)

                    # transpose one-hot back to channel-major
                    ps_oh = po.tile([128, TILE], BF16, tag="ohps")
                    for j in range(4):
                        nc.tensor.transpose(ps_oh[:, j * 128:(j + 1) * 128],
                                            eq_t[:, j * 128:(j + 1) * 128], idnbf[:])
                    oh_t = wp.tile([128, TILE], BF16, tag="oh")
                    nc.scalar.copy(oh_t[:], ps_oh[:])

                    # CondMul layer 1: all 8 experts, packed 4-per-matmul
                    ly = []
                    for g in range(2):
                        ps_y = pm.tile([128, TILE], F32, tag="mm")
                        nc.tensor.matmul(ps_y[:], w2p[g][0][:], rb_t[:],
                                         start=True, stop=False)
                        nc.tensor.matmul(ps_y[:], w2p[g][1][:], hb_t[:],
                                         start=False, stop=True)
                        ly_g = wp.tile([128, TILE], BF16, tag=f"ly{g}")
                        nc.scalar.activation(ly_g[:], ps_y[:], AF.Lrelu,
                                             bias=b2s[g][:], alpha=0.01)
                        ly.append(ly_g)

                    # gathered+expert-masked w3 via one-hot matmul, then product
                    mul = []
                    for g in range(2):
                        ps_w = pm.tile([128, TILE], F32, tag="mm")
                        nc.tensor.matmul(ps_w[:], w3sel[g][:], oh_t[:],
                                         start=True, stop=True)
                        mul_g = wp.tile([128, TILE], BF16, tag=f"mul{g}")
                        nc.vector.tensor_tensor(mul_g[:], ly[g][:], ps_w[:], ALU.mult)
                        mul.append(mul_g)

                    # result row: inds + b3[inds] + sum(y * w3g) , then /128
                    ps_res = pr.tile([1, TILE], F32, tag="rrow")
                    nc.tensor.matmul(ps_res[:], iotac[:], oh_t[:],
                                     start=True, stop=False)
                    nc.tensor.matmul(ps_res[:], b3col[:], oh_t[:],
                                     start=False, stop=False)
                    nc.tensor.matmul(ps_res[:], onesc[:], mul[0][:],
                                     start=False, stop=False)
                    nc.tensor.matmul(ps_res[:], onesc[:], mul[1][:],
                                     start=False, stop=True)
                    xrow = wp.tile([1, TILE], F32, tag="xrow")
                    nc.scalar.activation(xrow[:], ps_res[:], AF.Copy,
                                         scale=1.0 / CLASSES)
                    nc.sync.dma_start(xr_d[b:b + 1, w0:w0 + TILE], xrow[:])

    nc.compile()
    return nc


def _prep_consts(inputs):
    f32 = np.float32
    bf = ml_dtypes.bfloat16
    cl1_w = np.asarray(inputs['cl1_w'], f32)
    cl1_b = np.asarray(inputs['cl1_b'], f32)
    g1 = np.asarray(inputs['cl1_bn_g'], f32)
    bt1 = np.asarray(inputs['cl1_bn_b'], f32)
    m1 = np.asarray(inputs['cl1_bn_m'], f32)
    v1 = np.asarray(inputs['cl1_bn_v'], f32)
    cl2_w = np.asarray(inputs['cl2_w'], f32)
    cl2_b = np.asarray(inputs['cl2_b'], f32)
    cl3_w = np.asarray(inputs['cl3_w'], f32)
    cl3_b = np.asarray(inputs['cl3_b'], f32)
    reg1_w = np.asarray(inputs['reg1_w'], f32)
    reg1_b = np.asarray(inputs['reg1_b'], f32)
    gr = np.asarray(inputs['reg1_bn_g'], f32)
    btr = np.asarray(inputs['reg1_bn_b'], f32)
    mr = np.asarray(inputs['reg1_bn_m'], f32)
    vr = np.asarray(inputs['reg1_bn_v'], f32)
    w2 = np.asarray(inputs['w2'], f32)      # [8, 256, 32]
    b2 = np.asarray(inputs['b2'], f32)      # [8, 32]
    w3 = np.asarray(inputs['w3'], f32)      # [128, 32, 1]
    b3 = np.asarray(inputs['b3'], f32)      # [128, 1]

    s1 = g1 / np.sqrt(v1 + EPS)
    b1 = (cl1_b - m1) * s1 + bt1
    srv = gr / np.sqrt(vr + EPS)
    brv = (reg1_b - mr) * srv + btr

    # expert-packed CondMul weights: [g][kh][f, s*32+k] = w2[4g+s, kh*128+f, k]
    w2p = np.zeros((2, 2, 128, 128), f32)
    for g in range(2):
        for s in range(4):
            e = 4 * g + s
            for kh in range(2):
                w2p[g, kh, :, s * 32:(s + 1) * 32] = w2[e, kh * 128:(kh + 1) * 128, :]
    b2s = np.zeros((2, 128, 1), f32)
    for g in range(2):
        for s in range(4):
            b2s[g, s * 32:(s + 1) * 32, 0] = b2[4 * g + s]

    # block-masked w3 table: [g][c, s*32+k] = w3[c,k,0] if c//16 == 4g+s
    w3sel = np.zeros((2, 128, 128), f32)
    for c in range(128):
        e = c // 16
        g, s = divmod(e, 4)
        w3sel[g, c, s * 32:(s + 1) * 32] = w3[c, :, 0]

    return {
        "w1t": np.ascontiguousarray(cl1_w.T),
        "s1": s1.reshape(128, 1),
        "b1": b1.reshape(128, 1),
        "w2ct": np.ascontiguousarray(cl2_w.T),
        "b2c": cl2_b.reshape(128, 1),
        "w3ct": np.ascontiguousarray(cl3_w[:128].T),
        "b3c": cl3_b[:128].reshape(128, 1),
        "wlast": cl3_w[128].reshape(128, 1).astype(bf),
        "maskb": cl3_b[128].reshape(1, 1),
        "r1t": np.ascontiguousarray(reg1_w.T).astype(bf),
        "sr": srv.reshape(128, 1),
        "br": brv.reshape(128, 1),
        "w2p": w2p.astype(bf),
        "b2s": b2s,
        "w3sel": w3sel.astype(bf),
        "iotac": np.arange(128, dtype=f32).reshape(128, 1).astype(bf),
        "b3col": b3.reshape(128, 1).astype(bf),
        "onesc": np.ones((128, 1), f32).astype(bf),
        "idn32": np.eye(128, dtype=f32),
        "idnbf": np.eye(128, dtype=f32).astype(bf),
    }


def _run(inputs, trace=False, **kw):
    if "nc" not in _CACHE:
        _CACHE["nc"] = _build_nc()
    nc = _CACHE["nc"]

    consts = _prep_consts(inputs)
    x_in = np.asarray(inputs['x_in'], np.float32).reshape(B, C, W)

    in_maps = []
    for c in range(NCORES):
        m = dict(consts)
        m["x"] = np.ascontiguousarray(x_in[c * BPC:(c + 1) * BPC])
        in_maps.append(m)

    res = run_bass_kernel_spmd(nc, in_maps, list(range(NCORES)), trace=trace, **kw)

    xr = np.concatenate([res.results[c]["xr"] for c in range(NCORES)], axis=0)
    mask = np.concatenate([res.results[c]["mask"] for c in range(NCORES)], axis=0)
    out_xr = xr.reshape(B, 1, 1, W).astype(np.float32)
    out_mask = mask.reshape(B, 1, 1, W).astype(np.float32)
    return (out_xr, out_mask), res


def kernel(**inputs):
    (out_xr, out_mask), _ = _run(inputs)
    return (out_xr, out_mask)


# revision 6
# speedup vs baseline: 1.0386x; 1.0386x over previous
"""Trainium2 Bass kernel for nn_CR8_reg_cond_mul_5 (moe_routing).

Pipeline per pixel (B=16, C=128, H=1, W=8192; N = 131072 pixels):
  classifier: h = lrelu(bn(cl1 @ x)); x2 = lrelu(cl2 @ h); L = cl3 @ x2
  inds = argmax(L[:128]);  mask = lrelu(L[128])
  regression: r = lrelu(bn(reg1 @ x)); cat = [r; h]
  y = lrelu(cat @ w2[inds//16] + b2[inds//16])
  reg = y . w3[inds,:,0] + b3[inds];  x_real = (inds + reg) / 128

Sharding: data-parallel over batch; core c handles batches {2c, 2c+1}
(16384 pixels), weights replicated. No collectives.

On-chip strategy (channel-major [C, pixels] tiles of 512 px):
  - classifier matmuls in native fp32 (argmax needs fp32-grade logits);
  - argmax via PE transpose -> DVE max-reduce -> exact-equality one-hot
    -> PE transpose back to channel-major;
  - CondMul: all 8 experts computed as 2 expert-packed [128,...] bf16
    matmuls; per-pixel expert/class selection done by a single bf16
    matmul with a precomputed block-masked w3 table against the one-hot
    (folds the expert mask, the w3 gather and b3 gather into matmuls);
  - final dot + index + biases accumulated in one PSUM row.
"""
import numpy as np
import ml_dtypes

import concourse.bass as bass
import concourse.bacc as bacc
import concourse.mybir as mybir
import concourse.tile as tile
from concourse.bass_utils import run_bass_kernel_spmd

F32 = mybir.dt.float32
BF16 = mybir.dt.bfloat16
AF = mybir.ActivationFunctionType
ALU = mybir.AluOpType
AX = mybir.AxisListType

B, C, W = 16, 128, 8192
NCORES = 8
BPC = B // NCORES          # batches per core
TILE = 512
NTILES = W // TILE
CLASSES = 128
EPS = 1e-5

_CACHE = {}


def _build_nc(reps=1):
    nc = bacc.Bacc("TRN2", target_bir_lowering=False, debug=False)

    x_d = nc.dram_tensor("x", [BPC, C, W], F32, kind="ExternalInput")
    w1t_d = nc.dram_tensor("w1t", [128, 128], F32, kind="ExternalInput")
    s1_d = nc.dram_tensor("s1", [128, 1], F32, kind="ExternalInput")
    b1_d = nc.dram_tensor("b1", [128, 1], F32, kind="ExternalInput")
    w2ct_d = nc.dram_tensor("w2ct", [128, 128], F32, kind="ExternalInput")
    b2c_d = nc.dram_tensor("b2c", [128, 1], F32, kind="ExternalInput")
    w3ct_d = nc.dram_tensor("w3ct", [128, 128], F32, kind="ExternalInput")
    b3c_d = nc.dram_tensor("b3c", [128, 1], F32, kind="ExternalInput")
    wlast_d = nc.dram_tensor("wlast", [128, 1], BF16, kind="ExternalInput")
    maskb_d = nc.dram_tensor("maskb", [1, 1], F32, kind="ExternalInput")
    r1t_d = nc.dram_tensor("r1t", [128, 128], BF16, kind="ExternalInput")
    sr_d = nc.dram_tensor("sr", [128, 1], F32, kind="ExternalInput")
    br_d = nc.dram_tensor("br", [128, 1], F32, kind="ExternalInput")
    w2p_d = nc.dram_tensor("w2p", [2, 2, 128, 128], BF16, kind="ExternalInput")
    b2s_d = nc.dram_tensor("b2s", [2, 128, 1], F32, kind="ExternalInput")
    w3sel_d = nc.dram_tensor("w3sel", [2, 128, 128], BF16, kind="ExternalInput")
    iotac_d = nc.dram_tensor("iotac", [128, 1], BF16, kind="ExternalInput")
    b3col_d = nc.dram_tensor("b3col", [128, 1], BF16, kind="ExternalInput")
    onesc_d = nc.dram_tensor("onesc", [128, 1], BF16, kind="ExternalInput")
    idn32_d = nc.dram_tensor("idn32", [128, 128], F32, kind="ExternalInput")
    idnbf_d = nc.dram_tensor("idnbf", [128, 128], BF16, kind="ExternalInput")

    xr_d = nc.dram_tensor("xr", [BPC, W], F32, kind="ExternalOutput")
    mask_d = nc.dram_tensor("mask", [BPC, W], F32, kind="ExternalOutput")

    with tile.TileContext(nc) as tc:
        with (
            tc.tile_pool(name="consts", bufs=1) as cp,
            tc.tile_pool(name="xin", bufs=3) as xp,
            tc.tile_pool(name="work", bufs=3) as wp,
            tc.tile_pool(name="psmm", bufs=4, space="PSUM") as pm,
            tc.tile_pool(name="psoh", bufs=1, space="PSUM") as po,
            tc.tile_pool(name="psrow", bufs=1, space="PSUM") as pr,
        ):
            # ---- constants into SBUF once ----
            def cload(dram_ap, shape, dt, tag=None):
                t = cp.tile(shape, dt, tag=tag or dram_ap.tensor.name)
                nc.sync.dma_start(t[:], dram_ap)
                return t

            w1t = cload(w1t_d[:], [128, 128], F32)
            s1 = cload(s1_d[:], [128, 1], F32)
            b1 = cload(b1_d[:], [128, 1], F32)
            w2ct = cload(w2ct_d[:], [128, 128], F32)
            b2c = cload(b2c_d[:], [128, 1], F32)
            w3ct = cload(w3ct_d[:], [128, 128], F32)
            b3c = cload(b3c_d[:], [128, 1], F32)
            wlast = cload(wlast_d[:], [128, 1], BF16)
            maskb = cload(maskb_d[:], [1, 1], F32)
            r1t = cload(r1t_d[:], [128, 128], BF16)
            sr = cload(sr_d[:], [128, 1], F32)
            br = cload(br_d[:], [128, 1], F32)
            w2p = [[cload(w2p_d[g, kh], [128, 128], BF16, tag=f"w2p{g}{kh}") for kh in range(2)]
                   for g in range(2)]
            b2s = [cload(b2s_d[g], [128, 1], F32, tag=f"b2s{g}") for g in range(2)]
            w3sel = [cload(w3sel_d[g], [128, 128], BF16, tag=f"w3sel{g}") for g in range(2)]
            iotac = cload(iotac_d[:], [128, 1], BF16)
            b3col = cload(b3col_d[:], [128, 1], BF16)
            onesc = cload(onesc_d[:], [128, 1], BF16)
            idn32 = cload(idn32_d[:], [128, 128], F32)
            idnbf = cload(idnbf_d[:], [128, 128], BF16)

            for rep in range(reps):
             for b in range(BPC):
                for t in range(NTILES):
                    w0 = t * TILE
                    x_t = xp.tile([128, TILE], F32, tag="x")
                    nc.sync.dma_start(x_t[:], x_d[b, :, w0:w0 + TILE])
                    xb_t = wp.tile([128, TILE], BF16, tag="xb")
                    nc.vector.tensor_copy(xb_t[:], x_t[:])

                    # classifier layer 1 (fp32) + fused bnorm + lrelu
                    ps_h = pm.tile([128, TILE], F32, tag="mm")
                    nc.tensor.matmul(ps_h[:], w1t[:], x_t[:], start=True, stop=True)
                    h_t = wp.tile([128, TILE], F32, tag="h")
                    nc.scalar.activation(h_t[:], ps_h[:], AF.Lrelu,
                                         bias=b1[:], scale=s1[:], alpha=0.01)
                    hb_t = wp.tile([128, TILE], BF16, tag="hb")
                    nc.vector.tensor_copy(hb_t[:], h_t[:])

                    # regression layer 1 (bf16) + fused bnorm + lrelu
                    ps_r = pm.tile([128, TILE], F32, tag="mm")
                    nc.tensor.matmul(ps_r[:], r1t[:], xb_t[:], start=True, stop=True)
                    rb_t = wp.tile([128, TILE], BF16, tag="rb")
                    nc.scalar.activation(rb_t[:], ps_r[:], AF.Lrelu,
                                         bias=br[:], scale=sr[:], alpha=0.01)

                    # classifier layer 2 (fp32) + lrelu
                    ps_x2 = pm.tile([128, TILE], F32, tag="mm")
                    nc.tensor.matmul(ps_x2[:], w2ct[:], h_t[:], start=True, stop=True)
                    x2_t = wp.tile([128, TILE], F32, tag="x2")
                    nc.scalar.activation(x2_t[:], ps_x2[:], AF.Lrelu,
                                         bias=b2c[:], alpha=0.01)
                    x2b_t = wp.tile([128, TILE], BF16, tag="x2b")
                    nc.vector.tensor_copy(x2b_t[:], x2_t[:])
                    x2lo_t = wp.tile([128, TILE], BF16, tag="x2lo")
                    nc.vector.tensor_tensor(x2lo_t[:], x2_t[:], x2b_t[:], ALU.subtract)

                    # classifier layer 3 logits (fp32) + bias
                    ps_l = pm.tile([128, TILE], F32, tag="mm")
                    nc.tensor.matmul(ps_l[:], w3ct[:], x2_t[:], start=True, stop=True)
                    l_t = wp.tile([128, TILE], F32, tag="l")
                    nc.scalar.activation(l_t[:], ps_l[:], AF.Identity, bias=b3c[:])

                    # mask channel (bf16 row matmul) + lrelu
                    ps_m = pr.tile([1, TILE], F32, tag="mrow")
                    nc.tensor.matmul(ps_m[:], wlast[:], x2b_t[:], start=True, stop=False)
                    nc.tensor.matmul(ps_m[:], wlast[:], x2lo_t[:], start=False, stop=True)
                    mrow = wp.tile([1, TILE], F32, tag="mrow_sb")
                    nc.scalar.activation(mrow[:], ps_m[:], AF.Lrelu,
                                         bias=maskb[:], alpha=0.01)
                    nc.sync.dma_start(mask_d[b:b + 1, w0:w0 + TILE], mrow[:])

                    # transpose logits to pixel-major
                    ps_lt = pm.tile([128, TILE], F32, tag="mm")
                    for j in range(4):
                        nc.tensor.transpose(ps_lt[:, j * 128:(j + 1) * 128],
                                            l_t[:, j * 128:(j + 1) * 128], idn32[:])
                    lt3 = ps_lt[:].rearrange("p (b c) -> p b c", c=128)

                    # per-pixel max over classes, exact-equality one-hot
                    maxv = wp.tile([128, 4], F32, tag="maxv")
                    nc.vector.tensor_reduce(maxv[:], lt3, AX.X, ALU.max)
                    eq_t = wp.tile([128, TILE], BF16, tag="eq")
                    eq3 = eq_t[:].rearrange("p (b c) -> p b c", c=128)
                    maxb = maxv[:].unsqueeze(-1).broadcast_to([128, 4, 128])
                    nc.vector.tensor_tensor(eq3, lt3, maxb, ALU.is_equal)

                    # transpose one-hot back to channel-major
                    ps_oh = po.tile([128, TILE], BF16, tag="ohps")
                    for j in range(4):
                        nc.tensor.transpose(ps_oh[:, j * 128:(j + 1) * 128],
                                            eq_t[:, j * 128:(j + 1) * 128], idnbf[:])
                    oh_t = wp.tile([128, TILE], BF16, tag="oh")
                    nc.scalar.copy(oh_t[:], ps_oh[:])

                    # CondMul layer 1: all 8 experts, packed 4-per-matmul
                    ly = []
                    for g in range(2):
                        ps_y = pm.tile([128, TILE], F32, tag="mm")
                        nc.tensor.matmul(ps_y[:], w2p[g][0][:], rb_t[:],
                                         start=True, stop=False)
                        nc.tensor.matmul(ps_y[:], w2p[g][1][:], hb_t[:],
                                         start=False, stop=True)
                        ly_g = wp.tile([128, TILE], BF16, tag=f"ly{g}")
                        nc.scalar.activation(ly_g[:], ps_y[:], AF.Lrelu,
                                             bias=b2s[g][:], alpha=0.01)
                        ly.append(ly_g)

                    # gathered+expert-masked w3 via one-hot matmul, then product
                    mul = []
                    for g in range(2):
                        ps_w = pm.tile([128, TILE], F32, tag="mm")
                        nc.tensor.matmul(ps_w[:], w3sel[g][:], oh_t[:],
                                         start=True, stop=True)
                        mul_g = wp.tile([128, TILE], BF16, tag=f"mul{g}")
                        nc.vector.tensor_tensor(mul_g[:], ly[g][:], ps_w[:], ALU.mult)
                        mul.append(mul_g)

                    # result row: inds + b3[inds] + sum(y * w3g) , then /128
                    ps_res = pr.tile([1, TILE], F32, tag="rrow")
                    nc.tensor.matmul(ps_res[:], iotac[:], oh_t[:],
                                     start=True, stop=False)
                    nc.tensor.matmul(ps_res[:], b3col[:], oh_t[:],
                                     start=False, stop=False)
                    nc.tensor.matmul(ps_res[:], onesc[:], mul[0][:],
                                     start=False, stop=False)
                    nc.tensor.matmul(ps_res[:], onesc[:], mul[1][:],
                                     start=False, stop=True)
                    xrow = wp.tile([1, TILE], F32, tag="xrow")
                    nc.scalar.activation(xrow[:], ps_res[:], AF.Copy,
                                         scale=1.0 / CLASSES)
                    nc.sync.dma_start(xr_d[b:b + 1, w0:w0 + TILE], xrow[:])

    nc.compile()
    return nc


def _prep_consts(inputs):
    f32 = np.float32
    bf = ml_dtypes.bfloat16
    cl1_w = np.asarray(inputs['cl1_w'], f32)
    cl1_b = np.asarray(inputs['cl1_b'], f32)
    g1 = np.asarray(inputs['cl1_bn_g'], f32)
    bt1 = np.asarray(inputs['cl1_bn_b'], f32)
    m1 = np.asarray(inputs['cl1_bn_m'], f32)
    v1 = np.asarray(inputs['cl1_bn_v'], f32)
    cl2_w = np.asarray(inputs['cl2_w'], f32)
    cl2_b = np.asarray(inputs['cl2_b'], f32)
    cl3_w = np.asarray(inputs['cl3_w'], f32)
    cl3_b = np.asarray(inputs['cl3_b'], f32)
    reg1_w = np.asarray(inputs['reg1_w'], f32)
    reg1_b = np.asarray(inputs['reg1_b'], f32)
    gr = np.asarray(inputs['reg1_bn_g'], f32)
    btr = np.asarray(inputs['reg1_bn_b'], f32)
    mr = np.asarray(inputs['reg1_bn_m'], f32)
    vr = np.asarray(inputs['reg1_bn_v'], f32)
    w2 = np.asarray(inputs['w2'], f32)      # [8, 256, 32]
    b2 = np.asarray(inputs['b2'], f32)      # [8, 32]
    w3 = np.asarray(inputs['w3'], f32)      # [128, 32, 1]
    b3 = np.asarray(inputs['b3'], f32)      # [128, 1]

    s1 = g1 / np.sqrt(v1 + EPS)
    b1 = (cl1_b - m1) * s1 + bt1
    srv = gr / np.sqrt(vr + EPS)
    brv = (reg1_b - mr) * srv + btr

    # expert-packed CondMul weights: [g][kh][f, s*32+k] = w2[4g+s, kh*128+f, k]
    w2p = np.zeros((2, 2, 128, 128), f32)
    for g in range(2):
        for s in range(4):
            e = 4 * g + s
            for kh in range(2):
                w2p[g, kh, :, s * 32:(s + 1) * 32] = w2[e, kh * 128:(kh + 1) * 128, :]
    b2s = np.zeros((2, 128, 1), f32)
    for g in range(2):
        for s in range(4):
            b2s[g, s * 32:(s + 1) * 32, 0] = b2[4 * g + s]

    # block-masked w3 table: [g][c, s*32+k] = w3[c,k,0] if c//16 == 4g+s
    w3sel = np.zeros((2, 128, 128), f32)
    for c in range(128):
        e = c // 16
        g, s = divmod(e, 4)
        w3sel[g, c, s * 32:(s + 1) * 32] = w3[c, :, 0]

    return {
        "w1t": np.ascontiguousarray(cl1_w.T),
        "s1": s1.reshape(128, 1),
        "b1": b1.reshape(128, 1),
        "w2ct": np.ascontiguousarray(cl2_w.T),
        "b2c": cl2_b.reshape(128, 1),
        "w3ct": np.ascontiguousarray(cl3_w[:128].T),
        "b3c": cl3_b[:128].reshape(128, 1),
        "wlast": cl3_w[128].reshape(128, 1).astype(bf),
        "maskb": cl3_b[128].reshape(1, 1),
        "r1t": np.ascontiguousarray(reg1_w.T).astype(bf),
        "sr": srv.reshape(128, 1),
        "br": brv.reshape(128, 1),
        "w2p": w2p.astype(bf),
        "b2s": b2s,
        "w3sel": w3sel.astype(bf),
        "iotac": np.arange(128, dtype=f32).reshape(128, 1).astype(bf),
        "b3col": b3.reshape(128, 1).astype(bf),
        "onesc": np.ones((128, 1), f32).astype(bf),
        "idn32": np.eye(128, dtype=f32),
        "idnbf": np.eye(128, dtype=f32).astype(bf),
    }


def _run(inputs, trace=False, **kw):
    reps = kw.pop("reps", 1)
    key = ("nc", reps)
    if key not in _CACHE:
        _CACHE[key] = _build_nc(reps)
    nc = _CACHE[key]

    consts = _prep_consts(inputs)
    x_in = np.asarray(inputs['x_in'], np.float32).reshape(B, C, W)

    in_maps = []
    for c in range(NCORES):
        m = dict(consts)
        m["x"] = np.ascontiguousarray(x_in[c * BPC:(c + 1) * BPC])
        in_maps.append(m)

    res = run_bass_kernel_spmd(nc, in_maps, list(range(NCORES)), trace=trace, **kw)

    xr = np.concatenate([res.results[c]["xr"] for c in range(NCORES)], axis=0)
    mask = np.concatenate([res.results[c]["mask"] for c in range(NCORES)], axis=0)
    out_xr = xr.reshape(B, 1, 1, W).astype(np.float32)
    out_mask = mask.reshape(B, 1, 1, W).astype(np.float32)
    return (out_xr, out_mask), res


def kernel(**inputs):
    (out_xr, out_mask), _ = _run(inputs)
    return (out_xr, out_mask)


# revision 7
# speedup vs baseline: 8879.4724x; 8549.1611x over previous
"""Trainium2 Bass kernel for nn_CR8_reg_cond_mul_5 (moe_routing).

Pipeline per pixel (B=16, C=128, H=1, W=8192; N = 131072 pixels):
  classifier: h = lrelu(bn(cl1 @ x)); x2 = lrelu(cl2 @ h); L = cl3 @ x2
  inds = argmax(L[:128]);  mask = lrelu(L[128])
  regression: r = lrelu(bn(reg1 @ x)); cat = [r; h]
  y = lrelu(cat @ w2[inds//16] + b2[inds//16])
  reg = y . w3[inds,:,0] + b3[inds];  x_real = (inds + reg) / 128

Sharding: data-parallel over batch; core c handles batches {2c, 2c+1}
(16384 pixels), weights replicated. No collectives.

On-chip strategy (channel-major [C, pixels] tiles of 512 px):
  - classifier matmuls in native fp32 (argmax needs fp32-grade logits);
  - argmax via PE transpose -> DVE max-reduce -> exact-equality one-hot
    -> PE transpose back to channel-major;
  - CondMul: all 8 experts computed as 2 expert-packed [128,...] bf16
    matmuls; per-pixel expert/class selection done by a single bf16
    matmul with a precomputed block-masked w3 table against the one-hot
    (folds the expert mask, the w3 gather and b3 gather into matmuls);
  - final dot + index + biases accumulated in one PSUM row.
"""
import numpy as np
import ml_dtypes

import concourse.bass as bass
import concourse.bacc as bacc
import concourse.mybir as mybir
import concourse.tile as tile
from concourse.bass_utils import run_bass_kernel_spmd

F32 = mybir.dt.float32
BF16 = mybir.dt.bfloat16
AF = mybir.ActivationFunctionType
ALU = mybir.AluOpType
AX = mybir.AxisListType

B, C, W = 16, 128, 8192
NCORES = 8
BPC = B // NCORES          # batches per core
TILE = 512
NTILES = W // TILE
CLASSES = 128
EPS = 1e-5

_CACHE = {}


def _build_nc(reps=1):
    nc = bacc.Bacc("TRN2", target_bir_lowering=False, debug=False)

    x_d = nc.dram_tensor("x", [BPC, C, W], F32, kind="ExternalInput")
    w1t_d = nc.dram_tensor("w1t", [128, 128], F32, kind="ExternalInput")
    s1_d = nc.dram_tensor("s1", [128, 1], F32, kind="ExternalInput")
    b1_d = nc.dram_tensor("b1", [128, 1], F32, kind="ExternalInput")
    w2ct_d = nc.dram_tensor("w2ct", [128, 128], F32, kind="ExternalInput")
    b2c_d = nc.dram_tensor("b2c", [128, 1], F32, kind="ExternalInput")
    w3ct_d = nc.dram_tensor("w3ct", [128, 128], F32, kind="ExternalInput")
    b3c_d = nc.dram_tensor("b3c", [128, 1], F32, kind="ExternalInput")
    wlast_d = nc.dram_tensor("wlast", [128, 1], BF16, kind="ExternalInput")
    wlastlo_d = nc.dram_tensor("wlastlo", [128, 1], BF16, kind="ExternalInput")
    maskb_d = nc.dram_tensor("maskb", [1, 1], F32, kind="ExternalInput")
    r1t_d = nc.dram_tensor("r1t", [128, 128], BF16, kind="ExternalInput")
    sr_d = nc.dram_tensor("sr", [128, 1], F32, kind="ExternalInput")
    br_d = nc.dram_tensor("br", [128, 1], F32, kind="ExternalInput")
    w2p_d = nc.dram_tensor("w2p", [2, 2, 128, 128], BF16, kind="ExternalInput")
    b2s_d = nc.dram_tensor("b2s", [2, 128, 1], F32, kind="ExternalInput")
    w3sel_d = nc.dram_tensor("w3sel", [2, 128, 128], BF16, kind="ExternalInput")
    iotac_d = nc.dram_tensor("iotac", [128, 1], BF16, kind="ExternalInput")
    b3col_d = nc.dram_tensor("b3col", [128, 1], BF16, kind="ExternalInput")
    onesc_d = nc.dram_tensor("onesc", [128, 1], BF16, kind="ExternalInput")
    idn32_d = nc.dram_tensor("idn32", [128, 128], F32, kind="ExternalInput")
    idnbf_d = nc.dram_tensor("idnbf", [128, 128], BF16, kind="ExternalInput")

    xr_d = nc.dram_tensor("xr", [BPC, W], F32, kind="ExternalOutput")
    mask_d = nc.dram_tensor("mask", [BPC, W], F32, kind="ExternalOutput")

    with tile.TileContext(nc) as tc:
        with (
            tc.tile_pool(name="consts", bufs=1) as cp,
            tc.tile_pool(name="xin", bufs=3) as xp,
            tc.tile_pool(name="work", bufs=3) as wp,
            tc.tile_pool(name="psmm", bufs=4, space="PSUM") as pm,
            tc.tile_pool(name="psoh", bufs=1, space="PSUM") as po,
            tc.tile_pool(name="psrow", bufs=1, space="PSUM") as pr,
        ):
            # ---- constants into SBUF once ----
            def cload(dram_ap, shape, dt, tag=None):
                t = cp.tile(shape, dt, tag=tag or dram_ap.tensor.name)
                nc.sync.dma_start(t[:], dram_ap)
                return t

            w1t = cload(w1t_d[:], [128, 128], F32)
            s1 = cload(s1_d[:], [128, 1], F32)
            b1 = cload(b1_d[:], [128, 1], F32)
            w2ct = cload(w2ct_d[:], [128, 128], F32)
            b2c = cload(b2c_d[:], [128, 1], F32)
            w3ct = cload(w3ct_d[:], [128, 128], F32)
            b3c = cload(b3c_d[:], [128, 1], F32)
            wlast = cload(wlast_d[:], [128, 1], BF16)
            wlastlo = cload(wlastlo_d[:], [128, 1], BF16)
            maskb = cload(maskb_d[:], [1, 1], F32)
            r1t = cload(r1t_d[:], [128, 128], BF16)
            sr = cload(sr_d[:], [128, 1], F32)
            br = cload(br_d[:], [128, 1], F32)
            w2p = [[cload(w2p_d[g, kh], [128, 128], BF16, tag=f"w2p{g}{kh}") for kh in range(2)]
                   for g in range(2)]
            b2s = [cload(b2s_d[g], [128, 1], F32, tag=f"b2s{g}") for g in range(2)]
            w3sel = [cload(w3sel_d[g], [128, 128], BF16, tag=f"w3sel{g}") for g in range(2)]
            iotac = cload(iotac_d[:], [128, 1], BF16)
            b3col = cload(b3col_d[:], [128, 1], BF16)
            onesc = cload(onesc_d[:], [128, 1], BF16)
            idn32 = cload(idn32_d[:], [128, 128], F32)
            idnbf = cload(idnbf_d[:], [128, 128], BF16)

            for rep in range(reps):
             for b in range(BPC):
                for t in range(NTILES):
                    w0 = t * TILE
                    x_t = xp.tile([128, TILE], F32, tag="x")
                    nc.sync.dma_start(x_t[:], x_d[b, :, w0:w0 + TILE])
                    xb_t = wp.tile([128, TILE], BF16, tag="xb")
                    nc.vector.tensor_copy(xb_t[:], x_t[:])

                    # classifier layer 1 (fp32) + fused bnorm + lrelu
                    ps_h = pm.tile([128, TILE], F32, tag="mm")
                    nc.tensor.matmul(ps_h[:], w1t[:], x_t[:], start=True, stop=True)
                    h_t = wp.tile([128, TILE], F32, tag="h")
                    nc.scalar.activation(h_t[:], ps_h[:], AF.Lrelu,
                                         bias=b1[:], scale=s1[:], alpha=0.01)
                    hb_t = wp.tile([128, TILE], BF16, tag="hb")
                    nc.vector.tensor_copy(hb_t[:], h_t[:])

                    # regression layer 1 (bf16) + fused bnorm + lrelu
                    ps_r = pm.tile([128, TILE], F32, tag="mm")
                    nc.tensor.matmul(ps_r[:], r1t[:], xb_t[:], start=True, stop=True)
                    rb_t = wp.tile([128, TILE], BF16, tag="rb")
                    nc.scalar.activation(rb_t[:], ps_r[:], AF.Lrelu,
                                         bias=br[:], scale=sr[:], alpha=0.01)

                    # classifier layer 2 (fp32) + lrelu
                    ps_x2 = pm.tile([128, TILE], F32, tag="mm")
                    nc.tensor.matmul(ps_x2[:], w2ct[:], h_t[:], start=True, stop=True)
                    x2_t = wp.tile([128, TILE], F32, tag="x2")
                    nc.scalar.activation(x2_t[:], ps_x2[:], AF.Lrelu,
                                         bias=b2c[:], alpha=0.01)
                    x2b_t = wp.tile([128, TILE], BF16, tag="x2b")
                    nc.vector.tensor_copy(x2b_t[:], x2_t[:])
                    x2lo_t = wp.tile([128, TILE], BF16, tag="x2lo")
                    nc.vector.tensor_tensor(x2lo_t[:], x2_t[:], x2b_t[:], ALU.subtract)

                    # classifier layer 3 logits (fp32) + bias
                    ps_l = pm.tile([128, TILE], F32, tag="mm")
                    nc.tensor.matmul(ps_l[:], w3ct[:], x2_t[:], start=True, stop=True)
                    l_t = wp.tile([128, TILE], F32, tag="l")
                    nc.scalar.activation(l_t[:], ps_l[:], AF.Identity, bias=b3c[:])

                    # mask channel (bf16 row matmul) + lrelu
                    ps_m = pr.tile([1, TILE], F32, tag="mrow")
                    nc.tensor.matmul(ps_m[:], wlast[:], x2b_t[:], start=True, stop=False)
                    nc.tensor.matmul(ps_m[:], wlast[:], x2lo_t[:], start=False, stop=False)
                    nc.tensor.matmul(ps_m[:], wlastlo[:], x2b_t[:], start=False, stop=True)
                    mrow = wp.tile([1, TILE], F32, tag="mrow_sb")
                    nc.scalar.activation(mrow[:], ps_m[:], AF.Lrelu,
                                         bias=maskb[:], alpha=0.01)
                    nc.sync.dma_start(mask_d[b:b + 1, w0:w0 + TILE], mrow[:])

                    # transpose logits to pixel-major
                    ps_lt = pm.tile([128, TILE], F32, tag="mm")
                    for j in range(4):
                        nc.tensor.transpose(ps_lt[:, j * 128:(j + 1) * 128],
                                            l_t[:, j * 128:(j + 1) * 128], idn32[:])
                    lt3 = ps_lt[:].rearrange("p (b c) -> p b c", c=128)

                    # per-pixel max over classes, exact-equality one-hot
                    maxv = wp.tile([128, 4], F32, tag="maxv")
                    nc.vector.tensor_reduce(maxv[:], lt3, AX.X, ALU.max)
                    eq_t = wp.tile([128, TILE], BF16, tag="eq")
                    eq3 = eq_t[:].rearrange("p (b c) -> p b c", c=128)
                    maxb = maxv[:].unsqueeze(-1).broadcast_to([128, 4, 128])
                    nc.vector.tensor_tensor(eq3, lt3, maxb, ALU.is_equal)

                    # transpose one-hot back to channel-major
                    ps_oh = po.tile([128, TILE], BF16, tag="ohps")
                    for j in range(4):
                        nc.tensor.transpose(ps_oh[:, j * 128:(j + 1) * 128],
                                            eq_t[:, j * 128:(j + 1) * 128], idnbf[:])
                    oh_t = wp.tile([128, TILE], BF16, tag="oh")
                    nc.scalar.copy(oh_t[:], ps_oh[:])

                    # CondMul layer 1: all 8 experts, packed 4-per-matmul
                    ly = []
                    for g in range(2):
                        ps_y = pm.tile([128, TILE], F32, tag="mm")
                        nc.tensor.matmul(ps_y[:], w2p[g][0][:], rb_t[:],
                                         start=True, stop=False)
                        nc.tensor.matmul(ps_y[:], w2p[g][1][:], hb_t[:],
                                         start=False, stop=True)
                        ly_g = wp.tile([128, TILE], BF16, tag=f"ly{g}")
                        nc.scalar.activation(ly_g[:], ps_y[:], AF.Lrelu,
                                             bias=b2s[g][:], alpha=0.01)
                        ly.append(ly_g)

                    # gathered+expert-masked w3 via one-hot matmul, then product
                    mul = []
                    for g in range(2):
                        ps_w = pm.tile([128, TILE], F32, tag="mm")
                        nc.tensor.matmul(ps_w[:], w3sel[g][:], oh_t[:],
                                         start=True, stop=True)
                        mul_g = wp.tile([128, TILE], BF16, tag=f"mul{g}")
                        nc.vector.tensor_tensor(mul_g[:], ly[g][:], ps_w[:], ALU.mult)
                        mul.append(mul_g)

                    # result row: inds + b3[inds] + sum(y * w3g) , then /128
                    ps_res = pr.tile([1, TILE], F32, tag="rrow")
                    nc.tensor.matmul(ps_res[:], iotac[:], oh_t[:],
                                     start=True, stop=False)
                    nc.tensor.matmul(ps_res[:], b3col[:], oh_t[:],
                                     start=False, stop=False)
                    nc.tensor.matmul(ps_res[:], onesc[:], mul[0][:],
                                     start=False, stop=False)
                    nc.tensor.matmul(ps_res[:], onesc[:], mul[1][:],
                                     start=False, stop=True)
                    xrow = wp.tile([1, TILE], F32, tag="xrow")
                    nc.scalar.activation(xrow[:], ps_res[:], AF.Copy,
                                         scale=1.0 / CLASSES)
                    nc.sync.dma_start(xr_d[b:b + 1, w0:w0 + TILE], xrow[:])

    nc.compile()
    return nc


def _prep_consts(inputs):
    f32 = np.float32
    bf = ml_dtypes.bfloat16
    cl1_w = np.asarray(inputs['cl1_w'], f32)
    cl1_b = np.asarray(inputs['cl1_b'], f32)
    g1 = np.asarray(inputs['cl1_bn_g'], f32)
    bt1 = np.asarray(inputs['cl1_bn_b'], f32)
    m1 = np.asarray(inputs['cl1_bn_m'], f32)
    v1 = np.asarray(inputs['cl1_bn_v'], f32)
    cl2_w = np.asarray(inputs['cl2_w'], f32)
    cl2_b = np.asarray(inputs['cl2_b'], f32)
    cl3_w = np.asarray(inputs['cl3_w'], f32)
    cl3_b = np.asarray(inputs['cl3_b'], f32)
    reg1_w = np.asarray(inputs['reg1_w'], f32)
    reg1_b = np.asarray(inputs['reg1_b'], f32)
    gr = np.asarray(inputs['reg1_bn_g'], f32)
    btr = np.asarray(inputs['reg1_bn_b'], f32)
    mr = np.asarray(inputs['reg1_bn_m'], f32)
    vr = np.asarray(inputs['reg1_bn_v'], f32)
    w2 = np.asarray(inputs['w2'], f32)      # [8, 256, 32]
    b2 = np.asarray(inputs['b2'], f32)      # [8, 32]
    w3 = np.asarray(inputs['w3'], f32)      # [128, 32, 1]
    b3 = np.asarray(inputs['b3'], f32)      # [128, 1]

    s1 = g1 / np.sqrt(v1 + EPS)
    b1 = (cl1_b - m1) * s1 + bt1
    srv = gr / np.sqrt(vr + EPS)
    brv = (reg1_b - mr) * srv + btr

    # expert-packed CondMul weights: [g][kh][f, s*32+k] = w2[4g+s, kh*128+f, k]
    w2p = np.zeros((2, 2, 128, 128), f32)
    for g in range(2):
        for s in range(4):
            e = 4 * g + s
            for kh in range(2):
                w2p[g, kh, :, s * 32:(s + 1) * 32] = w2[e, kh * 128:(kh + 1) * 128, :]
    b2s = np.zeros((2, 128, 1), f32)
    for g in range(2):
        for s in range(4):
            b2s[g, s * 32:(s + 1) * 32, 0] = b2[4 * g + s]

    # block-masked w3 table: [g][c, s*32+k] = w3[c,k,0] if c//16 == 4g+s
    w3sel = np.zeros((2, 128, 128), f32)
    for c in range(128):
        e = c // 16
        g, s = divmod(e, 4)
        w3sel[g, c, s * 32:(s + 1) * 32] = w3[c, :, 0]

    return {
        "w1t": np.ascontiguousarray(cl1_w.T),
        "s1": s1.reshape(128, 1),
        "b1": b1.reshape(128, 1),
        "w2ct": np.ascontiguousarray(cl2_w.T),
        "b2c": cl2_b.reshape(128, 1),
        "w3ct": np.ascontiguousarray(cl3_w[:128].T),
        "b3c": cl3_b[:128].reshape(128, 1),
        "wlast": cl3_w[128].reshape(128, 1).astype(bf),
        "wlastlo": (cl3_w[128].reshape(128, 1)
                    - cl3_w[128].reshape(128, 1).astype(bf).astype(f32)).astype(bf),
        "maskb": cl3_b[128].reshape(1, 1),
        "r1t": np.ascontiguousarray(reg1_w.T).astype(bf),
        "sr": srv.reshape(128, 1),
        "br": brv.reshape(128, 1),
        "w2p": w2p.astype(bf),
        "b2s": b2s,
        "w3sel": w3sel.astype(bf),
        "iotac": np.arange(128, dtype=f32).reshape(128, 1).astype(bf),
        "b3col": b3.reshape(128, 1).astype(bf),
        "onesc": np.ones((128, 1), f32).astype(bf),
        "idn32": np.eye(128, dtype=f32),
        "idnbf": np.eye(128, dtype=f32).astype(bf),
    }


def _run(inputs, trace=False, **kw):
    reps = kw.pop("reps", 1)
    key = ("nc", reps)
    if key not in _CACHE:
        _CACHE[key] = _build_nc(reps)
    nc = _CACHE[key]

    consts = _prep_consts(inputs)
    x_in = np.asarray(inputs['x_in'], np.float32).reshape(B, C, W)

    in_maps = []
    for c in range(NCORES):
        m = dict(consts)
        m["x"] = np.ascontiguousarray(x_in[c * BPC:(c + 1) * BPC])
        in_maps.append(m)

    res = run_bass_kernel_spmd(nc, in_maps, list(range(NCORES)), trace=trace, **kw)

    xr = np.concatenate([res.results[c]["xr"] for c in range(NCORES)], axis=0)
    mask = np.concatenate([res.results[c]["mask"] for c in range(NCORES)], axis=0)
    out_xr = xr.reshape(B, 1, 1, W).astype(np.float32)
    out_mask = mask.reshape(B, 1, 1, W).astype(np.float32)
    return (out_xr, out_mask), res


def kernel(**inputs):
    (out_xr, out_mask), _ = _run(inputs)
    return (out_xr, out_mask)
